# revision 40
# baseline (speedup 1.0000x reference)
"""CGC multi-gate MoE kernel for Trainium2 (8 NeuronCores, data-parallel over batch).

Problem: 12 experts (4 shared / 4 task0 / 4 task1), each a 2-layer ReLU MLP
D=1024 -> H1=512 -> H2=256, over B=4096 rows; 3 softmax gates combine expert
outputs into t0/t1/ts [B, 256].

Strategy: pure batch data-parallel (512 rows/core, no collectives). Host
pre-transposes x (so the contraction dim D lands on SBUF partitions) and
pre-tiles weights into the exact SBUF layout for fully-contiguous DMA.
Layer-1 output is kept transposed on-chip ([H1, B] layout) so layer 2 needs no
transpose and its output lands with B on partitions, ready for the per-row
gate-weighted combine on DVE. Biases: b1 fused into the layer-1 ReLU copy on
ACT (per-partition bias); b2 added in-PSUM via a K=1 ones-row matmul.

Scheduling: x0 streams per-K-chunk on the scalar engine's DMA queue while
weights (and xs/x1, ordered by first use) stream on sync's; expert order is
task0 -> shared -> task1 so the first expert only needs x0, and every gate's
operands are resident before its matmuls come up in the PE stream. Dummy
warm-up matmuls hold the PE clock-gate (HAM) at 8/8 through the DMA-bound
ramp; combines for the first four experts defer until the gs gate exists.

Matmul dtype is selectable; default float16: same PE rate and DMA traffic as
bf16 (1 cycle/row, fast weight load) but 10-bit mantissas -> ~4e-4 rel err.
Also available: "bf16" (~3.5e-3), "f32r" (~2e-4, ~35% slower).
"""
import os
import sys

for _p in ("/opt/trn_rl_repo", "/root/.axon_site/_ro/trn_rl_repo"):
    if os.path.isdir(_p):
        if _p not in sys.path:
            sys.path.insert(0, _p)
        break

import numpy as np
from contextlib import ExitStack

import ml_dtypes

import concourse.bass as bass
import concourse.mybir as mybir
import concourse.tile as tile
from concourse import bacc
from concourse.bass_utils import run_bass_kernel_spmd

B, D, H1, H2 = 4096, 1024, 512, 256
NE = 12          # 4 shared + 4 task0 + 4 task1
NCORES = 8
BC = B // NCORES # 512 rows per core
P = 128
KO1 = D // P     # 8 contraction chunks, layer 1
KO2 = H1 // P    # 4 contraction chunks, layer 2
MT1 = H1 // P    # 4 output M-tiles, layer 1 (H1 on partitions)
BT = BC // P     # 4 B-tiles per core
GW = 28          # gate logit widths, concatenated: 8 (g0) + 8 (g1) + 12 (gs)

F32 = mybir.dt.float32
RELU = mybir.ActivationFunctionType.Relu
EXP = mybir.ActivationFunctionType.Exp
MULT = mybir.AluOpType.mult
ADD = mybir.AluOpType.add

# expert processing order: task0 (needs x0, first to arrive), shared, task1
EXPERT_ORDER = [4, 5, 6, 7, 0, 1, 2, 3, 8, 9, 10, 11]


def _build_program(dtype):
    MMD = {"bf16": mybir.dt.bfloat16, "fp16": mybir.dt.float16,
           "f32r": mybir.dt.float32r}[dtype]
    nc = bacc.Bacc("TRN2", target_bir_lowering=False, debug=False, num_devices=NCORES)

    xst = nc.dram_tensor("xst", [P, KO1, BC], MMD, kind="ExternalInput")
    x0t = nc.dram_tensor("x0t", [P, KO1, BC], MMD, kind="ExternalInput")
    x1t = nc.dram_tensor("x1t", [P, KO1, BC], MMD, kind="ExternalInput")
    w1 = nc.dram_tensor("w1", [NE, P, KO1, H1], MMD, kind="ExternalInput")
    w2 = nc.dram_tensor("w2", [NE, P, KO2, H2], MMD, kind="ExternalInput")
    wg = nc.dram_tensor("wg", [P, KO1, GW], MMD, kind="ExternalInput")
    b1d = nc.dram_tensor("b1d", [P, NE, MT1], F32, kind="ExternalInput")
    b2d = nc.dram_tensor("b2d", [1, NE, H2], MMD, kind="ExternalInput")
    onesd = nc.dram_tensor("onesd", [1, P], MMD, kind="ExternalInput")
    t0d = nc.dram_tensor("t0d", [P, BT, H2], F32, kind="ExternalOutput")
    t1d = nc.dram_tensor("t1d", [P, BT, H2], F32, kind="ExternalOutput")
    tsd = nc.dram_tensor("tsd", [P, BT, H2], F32, kind="ExternalOutput")

    with tile.TileContext(nc) as tc, ExitStack() as ctx:
        const = ctx.enter_context(tc.tile_pool(name="const", bufs=1))
        xpool = ctx.enter_context(tc.tile_pool(name="xpool", bufs=1))
        w1pool = ctx.enter_context(tc.tile_pool(name="w1pool", bufs=4))
        w2pool = ctx.enter_context(tc.tile_pool(name="w2pool", bufs=3))
        hpool = ctx.enter_context(tc.tile_pool(name="hpool", bufs=3))
        opool = ctx.enter_context(tc.tile_pool(name="opool", bufs=20))
        gtmp = ctx.enter_context(tc.tile_pool(name="gtmp", bufs=2))
        l1ps = ctx.enter_context(tc.tile_pool(name="l1ps", bufs=6, space="PSUM"))
        l2ps = ctx.enter_context(tc.tile_pool(name="l2ps", bufs=2, space="PSUM"))

        # tiny constants on the gpsimd (SWDGE) queue — keeps the two HWDGE
        # queues free for the x / weight streams from cycle one
        ones_sb = const.tile([1, P], MMD, name="ones_sb")
        nc.gpsimd.dma_start(ones_sb[:], onesd[:])
        wg_sb = const.tile([P, KO1, GW], MMD, name="wg_sb")
        nc.gpsimd.dma_start(wg_sb[:], wg[:])
        b1_sb = const.tile([P, NE, MT1], F32, name="b1_sb")
        nc.gpsimd.dma_start(b1_sb[:], b1d[:])
        b2_sb = const.tile([1, NE, H2], MMD, name="b2_sb")
        nc.gpsimd.dma_start(b2_sb[:], b2d[:])

        # x loads, chunked per K-slice on the vector engine's DMA queue, in
        # expert-consumption order: x0 first, xs second, x1 last.
        x0_sb = xpool.tile([P, KO1, BC], MMD, name="x0_sb")
        xs_sb = xpool.tile([P, KO1, BC], MMD, name="xs_sb")
        x1_sb = xpool.tile([P, KO1, BC], MMD, name="x1_sb")
        CHUNKS = [(0, 1), (1, 1), (2, 2), (4, 2), (6, 2)]
        for ko, n in CHUNKS:
            nc.scalar.dma_start(x0_sb[:, ko:ko + n, :], x0t[:, ko:ko + n, :])
        # xs / x1 are enqueued on the sync queue inside the expert loop, so
        # they stream AFTER the weights that are needed before them

        # HAM warm-up: ~3.5us of dummy PE activity in the window where the PE
        # would otherwise idle waiting for the first W1/x chunks, so the clock
        # gate is already at 8/8 when real matmuls start.
        if True:  # HAM warm-up block
            # full K=128 matmuls on scratch data (never read downstream), no
            # input deps -> they run right after the preamble and put the PE
            # clock gate at 8/8 before the first real matmul
            warm_w = const.tile([P, P], MMD, name="warm_w")
            nc.vector.memset(warm_w[:], 1.0)
            warm_ps = l2ps.tile([P, P], F32, tag="l2", name="warm_ps")
            for wi in range(32):
                nc.tensor.matmul(
                    warm_ps[:], warm_w[:], warm_w[:],
                    start=(wi == 0), stop=(wi == 31),
                )
            warm_out = gtmp.tile([P, 1], F32, tag="gn", name="warm_out")
            nc.vector.tensor_reduce(
                warm_out[:], warm_ps[:], axis=mybir.AxisListType.X,
                op=mybir.AluOpType.max,
            )

        g_sb = const.tile([P, BT, GW], F32, name="g_sb")
        t0a = const.tile([P, BT, H2], F32, name="t0a")
        t1a = const.tile([P, BT, H2], F32, name="t1a")
        tsa = const.tile([P, BT, H2], F32, name="tsa")
        acc_first = {id(a) * BT + bt: True
                     for a in (t0a, t1a, tsa) for bt in range(BT)}

        def emit_gate(src_sb, off, w):
            for bt in range(BT):
                psz = l2ps.tile([P, w], F32, tag="l2", name=f"psz_{off}_{bt}")
                for ko in range(KO1):
                    nc.tensor.matmul(
                        psz[:],
                        src_sb[:, ko, bt * P:(bt + 1) * P],
                        wg_sb[:, ko, off:off + w],
                        start=(ko == 0),
                        stop=(ko == KO1 - 1),
                    )
                nmax = gtmp.tile([P, 1], F32, tag="gn", name=f"nmax_{off}_{bt}")
                nc.vector.tensor_reduce(
                    nmax[:], psz[:], axis=mybir.AxisListType.X,
                    op=mybir.AluOpType.max, negate=True,
                )
                e_sb = gtmp.tile([P, w], F32, tag="ge", name=f"e_sb_{off}_{bt}")
                nc.scalar.activation(e_sb[:], psz[:], EXP, bias=nmax[:], scale=1.0)
                ssum = gtmp.tile([P, 1], F32, tag="gs", name=f"ssum_{off}_{bt}")
                nc.vector.tensor_reduce(
                    ssum[:], e_sb[:], axis=mybir.AxisListType.X, op=ADD,
                )
                rsum = gtmp.tile([P, 1], F32, tag="gr", name=f"rsum_{off}_{bt}")
                nc.vector.reciprocal(rsum[:], ssum[:])
                nc.vector.tensor_scalar_mul(g_sb[:, bt, off:off + w], e_sb[:], rsum[:])

        def expert_targets(e):
            if e < 4:
                return [(t0a, 0 + e), (t1a, 8 + e), (tsa, 16 + e)]
            if e < 8:
                return [(t0a, 0 + e), (tsa, 16 + e)]
            return [(t1a, 8 + 4 + (e - 8)), (tsa, 16 + e)]

        def combine_bt(e, idx, bt, o_sb):
            for acc, col in expert_targets(e):
                sc = g_sb[:, bt, col:col + 1]
                if acc_first[id(acc) * BT + bt]:
                    acc_first[id(acc) * BT + bt] = False
                    nc.vector.tensor_scalar_mul(acc[:, bt, :], o_sb[:], sc)
                else:
                    nc.vector.scalar_tensor_tensor(
                        acc[:, bt, :], o_sb[:], sc, acc[:, bt, :],
                        op0=MULT, op1=ADD,
                    )
            # drain finished output rows as soon as their last
            # contribution lands (t0: after e4..7+e0..3 = idx 7)
            if idx == 7:
                nc.sync.dma_start(t0d[:, bt, :], t0a[:, bt, :])
            elif idx == 11:
                nc.sync.dma_start(t1d[:, bt, :], t1a[:, bt, :])
                nc.sync.dma_start(tsd[:, bt, :], tsa[:, bt, :])

        deferred = []  # (e, idx, o_tiles) for idx<=DEFER_LAST, combined after gs
        DEFER_LAST = 3
        for idx, e in enumerate(EXPERT_ORDER):
            src_sb = xs_sb if e < 4 else (x0_sb if e < 8 else x1_sb)

            w1_sb = w1pool.tile([P, KO1, H1], MMD, tag="w1", name=f"w1_sb_{e}")
            if idx < 2:
                # chunked so matmuls can start before the full tile lands;
                # very first piece halved again to beat the cold-queue latency
                if idx == 0:
                    nc.sync.dma_start(w1_sb[:, 0, :256], w1[e, :, 0, :256])
                    nc.sync.dma_start(w1_sb[:, 0, 256:], w1[e, :, 0, 256:])
                else:
                    nc.sync.dma_start(w1_sb[:, 0, :], w1[e, :, 0, :])
                for ko, n in CHUNKS[1:]:
                    nc.sync.dma_start(w1_sb[:, ko:ko + n, :], w1[e, :, ko:ko + n, :])
            else:
                nc.sync.dma_start(w1_sb[:], w1[e])
            w2_sb = w2pool.tile([P, KO2, H2], MMD, tag="w2", name=f"w2_sb_{e}")
            nc.sync.dma_start(w2_sb[:], w2[e])
            if idx == 1:
                nc.sync.dma_start(xs_sb[:], xst[:])
            elif idx == 2:
                nc.sync.dma_start(x1_sb[:], x1t[:])

            # layer 1: hT[H1, BC] = relu(W1[e].T-chunks @ xT + b1[e])
            hT = hpool.tile([P, MT1, BC], MMD, tag="h", name=f"hT_{e}")
            phs = [l1ps.tile([P, BC], F32, tag="l1", name=f"ph_{e}_{m}")
                   for m in range(MT1)]
            if idx < 2:
                # ko-major: each arriving W1/x chunk feeds MT1 matmuls, so the
                # PE keeps pace with the startup DMA stream
                for ko in range(KO1):
                    for m in range(MT1):
                        nc.tensor.matmul(
                            phs[m][:],
                            w1_sb[:, ko, m * P:(m + 1) * P],
                            src_sb[:, ko, :],
                            start=(ko == 0),
                            stop=(ko == KO1 - 1),
                        )
            else:
                for m in range(MT1):
                    for ko in range(KO1):
                        nc.tensor.matmul(
                            phs[m][:],
                            w1_sb[:, ko, m * P:(m + 1) * P],
                            src_sb[:, ko, :],
                            start=(ko == 0),
                            stop=(ko == KO1 - 1),
                        )
            for m in range(MT1):
                nc.scalar.activation(
                    hT[:, m, :], phs[m][:], RELU, bias=b1_sb[:, e, m:m + 1], scale=1.0,
                )

            # gates go into the PE stream right when their operands are resident
            GS_IDX, G1_IDX = 3, 4
            if idx == 0:
                emit_gate(x0_sb, 0, 8)    # g0
            elif idx == GS_IDX:
                emit_gate(xs_sb, 16, 12)  # gs (xs resident by now)
            elif idx == G1_IDX:
                emit_gate(x1_sb, 8, 8)    # g1

            # layer 2 + b2 (K=1 ones-row matmul) + relu + combine, per B-tile
            o_tiles = []
            for bt in range(BT):
                po = l2ps.tile([P, H2], F32, tag="l2", name=f"po_{e}_{bt}")
                for kh in range(KO2):
                    nc.tensor.matmul(
                        po[:],
                        hT[:, kh, bt * P:(bt + 1) * P],
                        w2_sb[:, kh, :],
                        start=(kh == 0),
                        stop=False,
                    )
                nc.tensor.matmul(
                    po[:], ones_sb[:1, :P], b2_sb[:1, e, :], start=False, stop=True,
                )
                o_sb = opool.tile([P, H2], F32, tag="o", name=f"o_{e}_{bt}")
                nc.scalar.activation(o_sb[:], po[:], RELU)
                o_tiles.append(o_sb)
                if idx > DEFER_LAST:
                    combine_bt(e, idx, bt, o_sb)

            if idx < DEFER_LAST:
                # combines for the deferred experts need gs
                deferred.append((e, idx, o_tiles))
            elif idx == DEFER_LAST:
                deferred.append((e, idx, o_tiles))
                for de, didx, dtiles in deferred:
                    for bt in range(BT):
                        combine_bt(de, didx, bt, dtiles[bt])
                deferred = None

    nc.finalize()
    return nc


_PROGRAMS = {}


def _get_program(dtype):
    if dtype not in _PROGRAMS:
        _PROGRAMS[dtype] = _build_program(dtype)
    return _PROGRAMS[dtype]


def _prep_inputs(x0, x1, xs, W1, b1, W2, b2, Wg0, Wg1, Wgs, dtype):
    """Host-side shard + relayout into the DMA-friendly per-core layouts."""
    f = np.float32
    md = {"bf16": ml_dtypes.bfloat16, "fp16": np.float16, "f32r": np.float32}[dtype]

    def xt_core(x, c):
        # x [B, D] -> core slice transposed/tiled to [P, KO1, BC]
        s = np.asarray(x[c * BC:(c + 1) * BC], f).T          # [D, BC]
        return np.ascontiguousarray(
            s.reshape(KO1, P, BC).transpose(1, 0, 2).astype(md))

    w1r = np.ascontiguousarray(
        np.asarray(W1, f).reshape(NE, KO1, P, H1).transpose(0, 2, 1, 3).astype(md))
    w2r = np.ascontiguousarray(
        np.asarray(W2, f).reshape(NE, KO2, P, H2).transpose(0, 2, 1, 3).astype(md))
    wgr = np.ascontiguousarray(
        np.concatenate([np.asarray(Wg0, f), np.asarray(Wg1, f), np.asarray(Wgs, f)],
                       axis=1).reshape(KO1, P, GW).transpose(1, 0, 2).astype(md))
    b1r = np.ascontiguousarray(np.asarray(b1, f).reshape(NE, MT1, P).transpose(2, 0, 1))
    b2r = np.ascontiguousarray(np.asarray(b2, f).reshape(1, NE, H2).astype(md))
    ones = np.ones((1, P), md)

    in_maps = []
    for c in range(NCORES):
        in_maps.append({
            "xst": xt_core(xs, c),
            "x0t": xt_core(x0, c),
            "x1t": xt_core(x1, c),
            "w1": w1r,
            "w2": w2r,
            "wg": wgr,
            "b1d": b1r,
            "b2d": b2r,
            "onesd": ones,
        })
    return in_maps


def _assemble(results):
    outs = []
    for name in ("t0d", "t1d", "tsd"):
        parts = [
            results[c][name].transpose(1, 0, 2).reshape(BC, H2)
            for c in range(NCORES)
        ]
        outs.append(np.ascontiguousarray(np.concatenate(parts, axis=0)))
    return tuple(outs)


def kernel(x0, x1, xs, W1, b1, W2, b2, Wg0, Wg1, Wgs, dtype="fp16", **run_kwargs):
    nc = _get_program(dtype)
    in_maps = _prep_inputs(x0, x1, xs, W1, b1, W2, b2, Wg0, Wg1, Wgs, dtype)
    res = run_bass_kernel_spmd(nc, in_maps, core_ids=list(range(NCORES)), **run_kwargs)
    out = _assemble(res.results)
    if run_kwargs:
        return out, res
    return out
